# revision 19
# baseline (speedup 1.0000x reference)
"""Trainium2 Bass kernel for a dense transformer layer (attention + FFN).

Sharding: 8 shards = (batch b, sequence half) pairs. Each core computes the
full K/V projections for its batch (2x redundant) and Q/attention/FFN for its
1024-token query slice. No cross-core communication.

On-device layout is feature-major (transposed): activations live as
[feature, token] so every matmul is lhsT.T @ rhs with natural weight layouts.

Precision: QKV/O projections, the attention ctx matmuls, and BOTH FFN
matmuls run fp8e4 with DoubleRow perf mode (256-row contraction per
instruction, ~1.9x PE speedup). The FFN stays inside the 2e-2 error gate
via a gelu linear-split: gelu(z) = ALPHA*z + h'(z) with h' = gelu - ALPHA*z.
The ALPHA*z branch is exact linear algebra folded into a host-precomputed
merged matrix M = ALPHA*W1@W2 applied to y in bf16 (8 single-row matmuls
per output chunk); only the small-magnitude h' branch runs fp8, cutting the
fp8 FFN2 quantization error ~3x (sim: full-fp8 direct 2.5e-2; split 1.5e-2).
Weights are pre-scaled x16 on the host to sit in fp8e4's normal range; the
1/16 is folded into the PSUM->SBUF readout ops. ctx is scaled x32 via the
softmax reciprocal (compensated in Wo). fp32 PSUM accumulation; residual
stream held bf16 (acc16).

Schedule: the query slice is split in two 512-token halves. Phase A runs
K/Q/V projections and half-0 attention (ACT-bound on exp). Phase B runs
half-1 attention on ACT while the PE stream interleaves half-0's O
projection and FFN between attention matmuls — keeping the PE array dense
(avoids HAM down-throttle) and overlapping the exp floor with FFN compute.
Phase C finishes half-1's O projection and FFN. W1 (4MB fp8) is resident in
SBUF from phase A; W2/M stream in small fp8/bf16 chunks.
"""

import numpy as np
import ml_dtypes

B, S, D = 4, 2048, 1024
H, DH, F = 16, 64, 4096
P = 128
NCORES = 8
SQ = B * S // NCORES  # 1024 query tokens per core
HQ = SQ // 2  # 512-token query half
DC = D // P  # 8 feature chunks
DC2 = DC // 2  # 4 double chunks (DoubleRow)
FC = F // P  # 32 ffn chunks
FC2 = FC // 2
SKC = S // P  # 16 key chunks
SKC2 = SKC // 2
NPAIR = H // 2  # 8 head pairs (2 heads per 128-feature chunk)
GB = 8  # gelu batch (chunks per ACT gelu instruction)

WS = 16.0  # host-side fp8 weight scale
RWS = 1.0 / WS
CTXS = 32.0  # ctx fp8 scale (folded into softmax recip; compensated in Wo)
ALPHA = 0.6  # gelu linear-split coefficient (min error on this data)

BF16 = ml_dtypes.bfloat16
FP8 = ml_dtypes.float8_e4m3

_CACHE = {}


def _build_program():
    import concourse.mybir as mybir
    import concourse.tile as tile
    from concourse import bacc

    f32 = mybir.dt.float32
    bf16 = mybir.dt.bfloat16
    fp8 = mybir.dt.float8e4
    AF = mybir.ActivationFunctionType
    DR = mybir.MatmulPerfMode.DoubleRow
    MUL = mybir.AluOpType.mult
    ADD = mybir.AluOpType.add

    nc = bacc.Bacc("TRN2", target_bir_lowering=False, debug=False, num_devices=NCORES)

    # xT/wv/wk/wq are chunk-major so every startup DMA moves contiguous
    # 1-4KB per-partition lines (small strided lines gut DMA throughput)
    xT_d = nc.dram_tensor("xT", [P, 4, DC, 512], fp8, kind="ExternalInput")
    xres_d = nc.dram_tensor("xres", [P, DC, SQ], f32, kind="ExternalInput")
    wq_d = nc.dram_tensor("wq", [P, NPAIR, DC, P], fp8, kind="ExternalInput")
    wk_d = nc.dram_tensor("wk", [P, NPAIR, DC, P], fp8, kind="ExternalInput")
    wv_d = nc.dram_tensor("wv", [P, 2, DC, 512], fp8, kind="ExternalInput")
    wo_d = nc.dram_tensor("wo", [P, DC, D], fp8, kind="ExternalInput")
    w1_d = nc.dram_tensor("w1", [P, FC, DC, P], fp8, kind="ExternalInput")
    w2_d = nc.dram_tensor("w2", [DC, P, FC, P], fp8, kind="ExternalInput")
    m_d = nc.dram_tensor("m", [DC, P, DC, P], bf16, kind="ExternalInput")
    bq_d = nc.dram_tensor("bq", [P, DC], f32, kind="ExternalInput")
    bk_d = nc.dram_tensor("bk", [P, DC], f32, kind="ExternalInput")
    bvb_d = nc.dram_tensor("bvb", [P, D], bf16, kind="ExternalInput")
    b1_d = nc.dram_tensor("b1", [P, FC], f32, kind="ExternalInput")
    b2_d = nc.dram_tensor("b2", [P, DC], f32, kind="ExternalInput")
    outT_d = nc.dram_tensor("outT", [P, DC, SQ], f32, kind="ExternalOutput")

    with tile.TileContext(nc) as tc:
        with (
            tc.tile_pool(name="psA", bufs=2, space="PSUM") as psA,
            tc.tile_pool(name="psS", bufs=2, space="PSUM") as psS,
            tc.tile_pool(name="psC", bufs=2, space="PSUM") as psC,
            tc.tile_pool(name="biasp", bufs=1) as biasp,
            tc.tile_pool(name="ctxp", bufs=1) as ctxp,
            tc.tile_pool(name="ep", bufs=6) as ep,
            tc.tile_pool(name="rp", bufs=2) as rp,
            tc.tile_pool(name="rbp", bufs=2) as rbp,
        ):
            bq_sb = biasp.tile([P, DC], f32)
            bk_sb = biasp.tile([P, DC], f32)
            b1_sb = biasp.tile([P, FC], f32)
            b2_sb = biasp.tile([P, DC], f32)
            nc.scalar.dma_start(bq_sb[:], bq_d[:])
            nc.scalar.dma_start(bk_sb[:], bk_d[:])
            nc.scalar.dma_start(b1_sb[:], b1_d[:])
            nc.scalar.dma_start(b2_sb[:], b2_d[:])

            ctxT_sb = ctxp.tile([P, DC, SQ], fp8)
            wo_sb = ctxp.tile([P, DC, D], fp8)
            w1_sb = ctxp.tile([P, FC, DC, P], fp8)
            v_sb = ctxp.tile([P, SKC, H, DH + 1], fp8)
            kt_all = ctxp.tile([P, NPAIR, S], fp8)
            qt_all = ctxp.tile([P, NPAIR, SQ], fp8)

            def attn_pair(p, half, spread_hook=None):
                """Attention for head pair (2p, 2p+1), queries
                [half*HQ, half*HQ+HQ). spread_hook(skh) emits filler PE work.

                Software-pipelined: scores run one sk-chunk-pair ahead of ctx
                so the in-order PE stream never serializes the next scores
                behind exp — ACT stays continuously busy on exp."""
                q0 = half * HQ
                pc0 = psC.tile([P, HQ], f32, tag="pc")
                pc1 = psC.tile([P, HQ], f32, tag="pc")
                E2s = [None] * SKC2

                def emit_scores(skh):
                    # E2: exp(scores), laid [key, chunk-parity, headA|headB]
                    # = the ctx DoubleRow moving operand.
                    E2 = ep.tile([P, 2, 2 * HQ], fp8)
                    E2s[skh] = E2
                    for hs in range(2):
                        sk = 2 * skh + hs
                        ss = psS.tile([P, 2 * HQ], f32)
                        nc.tensor.matmul(
                            ss[:, 0:HQ],
                            kt_all[0:64, p, sk * P : (sk + 1) * P],
                            qt_all[0:64, p, q0 : q0 + HQ],
                            start=True,
                            stop=True,
                        )
                        nc.tensor.matmul(
                            ss[:, HQ : 2 * HQ],
                            kt_all[64:128, p, sk * P : (sk + 1) * P],
                            qt_all[64:128, p, q0 : q0 + HQ],
                            start=True,
                            stop=True,
                        )
                        nc.scalar.activation(E2[:, hs, :], ss, AF.Exp)

                emit_scores(0)
                for skh in range(SKC2):
                    if skh + 1 < SKC2:
                        emit_scores(skh + 1)
                    # filler PE work lands between next-scores and this ctx so
                    # the PE covers the exp latency instead of stalling on E2
                    if spread_hook is not None:
                        spread_hook(skh)
                    E2 = E2s[skh]
                    nc.tensor.matmul(
                        pc0[:65],
                        v_sb[:, 2 * skh : 2 * skh + 2, 2 * p, :],
                        E2[:, :, 0:HQ],
                        start=(skh == 0),
                        stop=(skh == SKC2 - 1),
                        perf_mode=DR,
                    )
                    nc.tensor.matmul(
                        pc1[:65],
                        v_sb[:, 2 * skh : 2 * skh + 2, 2 * p + 1, :],
                        E2[:, :, HQ : 2 * HQ],
                        start=(skh == 0),
                        stop=(skh == SKC2 - 1),
                        perf_mode=DR,
                    )
                # softmax normalization: ctx * (CTXS / rowsum); the CTXS fp8
                # range scale is divided back out in Wo. (approx recip is ~18
                # correct bits, plenty for a softmax denom)
                for hh, pc in ((0, pc0), (1, pc1)):
                    s0 = rp.tile([1, HQ], f32, tag="s")
                    nc.vector.tensor_scalar_mul(s0, pc[64:65, :], 1.0 / CTXS)
                    r0 = rp.tile([1, HQ], f32, tag="r")
                    nc.vector.reciprocal_approx_fast(r0, s0)
                    rb0 = rbp.tile([64, HQ], f32, tag="rb")
                    nc.gpsimd.partition_broadcast(rb0, r0)
                    nc.vector.tensor_mul(
                        ctxT_sb[64 * hh : 64 * hh + 64, p, q0 : q0 + HQ],
                        pc[0:64, :],
                        rb0,
                    )

            # ---------------- Phase A: projections + half-0 attention -------
            with (
                tc.tile_pool(name="abp", bufs=1) as abp,
                tc.tile_pool(name="wvp", bufs=1) as wvp,
                tc.tile_pool(name="ws", bufs=3) as ws,
            ):
                # x^T in 4 column-chunk tiles so V/K matmuls start after the
                # first chunk lands rather than after the full DMA.
                xTs = [
                    abp.tile([P, DC, 512], fp8, tag=f"xT{c}", name=f"xT{c}")
                    for c in range(4)
                ]
                wvs = [
                    wvp.tile([P, DC, 512], fp8, tag=f"wv{c}", name=f"wv{c}")
                    for c in range(2)
                ]
                bvb_sb = abp.tile([P, D], bf16)
                # startup DMA priority comes from in-queue FIFO order: the
                # first V matmul's inputs (x and wv chunks 0-1) lead their
                # queues; bulk transfers follow behind them, spread over the
                # sync/gpsimd/scalar queues.
                nc.sync.dma_start(xTs[0][:, 0:2, :], xT_d[:, 0, 0:2])
                nc.gpsimd.dma_start(wvs[0][:, 0:2, :], wv_d[:, 0, 0:2])
                nc.sync.dma_start(xTs[0][:, 2:DC, :], xT_d[:, 0, 2:DC])
                nc.gpsimd.dma_start(wvs[0][:, 2:DC, :], wv_d[:, 0, 2:DC])
                nc.scalar.dma_start(bvb_sb[:], bvb_d[:])
                nc.sync.dma_start(xTs[1][:], xT_d[:, 1])
                nc.scalar.dma_start(xTs[2][:], xT_d[:, 2])
                nc.sync.dma_start(xTs[3][:], xT_d[:, 3])
                nc.gpsimd.dma_start(wvs[1][:], wv_d[:, 1])
                nc.scalar.dma_start(wo_sb[:], wo_d[:])
                # resident full-fp8 W1 (4MB) in two big contiguous DMAs on
                # the gpsimd queue (NOT scalar: scalar-queue DMA triggers
                # would steal ACT cycles from the exp stream)
                nc.gpsimd.dma_start(w1_sb[:, 0 : FC // 2], w1_d[:, 0 : FC // 2])
                nc.gpsimd.dma_start(w1_sb[:, FC // 2 :], w1_d[:, FC // 2 :])

                # V projection, token-major: v[sk, dv] (+ ones column per
                # head). fp8: it is the ctx DoubleRow stationary operand.
                nc.vector.memset(v_sb[:, :, :, DH : DH + 1], 1.0)

                def emit_v(nv, sks, h0=0, h1=8):
                    nh = h1 - h0
                    for sk in sks:
                        xt = xTs[sk // 4]
                        co = (sk % 4) * P
                        ps = psA.tile([P, 512], f32, tag="ps")
                        for c in range(DC2):
                            nc.tensor.matmul(
                                ps[:, : nh * DH],
                                xt[:, 2 * c : 2 * c + 2, co : co + P],
                                wvs[nv][:, 2 * c : 2 * c + 2, h0 * DH : h1 * DH],
                                start=(c == 0),
                                stop=(c == DC2 - 1),
                                perf_mode=DR,
                            )
                        nc.vector.scalar_tensor_tensor(
                            v_sb[:, sk, nv * 8 + h0 : nv * 8 + h1, 0:DH],
                            ps[:, : nh * DH].rearrange("p (h d) -> p h d", h=nh),
                            RWS,
                            bvb_sb[
                                :, nv * 512 + h0 * DH : nv * 512 + h1 * DH
                            ].rearrange("p (h d) -> p h d", h=nh),
                            MUL,
                            ADD,
                        )

                def kq_units(p):
                    """K/Q projection PE work for pair p as 6 ~1-1.5us units.
                    Weight DMAs are issued at queue-build time (prefetch)."""
                    wkt = ws.tile([P, DC, P], fp8, tag="wchunk")
                    nc.sync.dma_start(wkt[:], wk_d[:, p])
                    wqt = ws.tile([P, DC, P], fp8, tag="wchunk")
                    nc.sync.dma_start(wqt[:], wq_d[:, p])
                    units = []
                    for n in range(S // 512):
                        def ku(n=n):
                            ps = psA.tile([P, 512], f32, tag="ps")
                            for c in range(DC2):
                                nc.tensor.matmul(
                                    ps,
                                    wkt[:, 2 * c : 2 * c + 2, :],
                                    xTs[n][:, 2 * c : 2 * c + 2, :],
                                    start=(c == 0),
                                    stop=(c == DC2 - 1),
                                    perf_mode=DR,
                                )
                            nc.vector.tensor_scalar(
                                kt_all[:, p, n * 512 : (n + 1) * 512],
                                ps,
                                RWS,
                                bk_sb[:, p : p + 1],
                                MUL,
                                ADD,
                            )
                        units.append(ku)
                    # wq is x16 overall on the host (x128 on Wq*scale for fp8
                    # range); divide the full 128 back out in the readout.
                    # xT is host-rotated per core so its own 1024 query
                    # tokens are chunks 0-1 (keys permuted; softmax invariant)
                    for n in range(SQ // 512):
                        def qu(n=n):
                            ps = psA.tile([P, 512], f32, tag="ps")
                            for c in range(DC2):
                                nc.tensor.matmul(
                                    ps,
                                    wqt[:, 2 * c : 2 * c + 2, :],
                                    xTs[n][:, 2 * c : 2 * c + 2, :],
                                    start=(c == 0),
                                    stop=(c == DC2 - 1),
                                    perf_mode=DR,
                                )
                            nc.vector.tensor_scalar(
                                qt_all[:, p, n * 512 : (n + 1) * 512],
                                ps,
                                RWS / 8.0,
                                bq_sb[:, p : p + 1],
                                MUL,
                                ADD,
                            )
                        units.append(qu)
                    return units

                # V(nv=1) chunk counts per pair (heads 8-15, needed from
                # pair 4 on — must complete by end of pair 3)
                V1_PLAN = {0: 2, 1: 5, 2: 5, 3: 4}

                emit_v(0, range(SKC))
                for u in kq_units(0):
                    u()
                v1_next = 0
                for p in range(NPAIR):
                    queue = []
                    if p + 1 < NPAIR:
                        queue.extend(kq_units(p + 1))
                    for _ in range(V1_PLAN.get(p, 0)):
                        queue.append(lambda sk=v1_next: emit_v(1, [sk]))
                        v1_next += 1

                    def hook(skh, queue=queue):
                        # drain >=1 unit/slot, catching up so the queue
                        # empties by the last slot of the pair
                        rem_slots = SKC2 - skh
                        n = max(1, -(-len(queue) // rem_slots))
                        for _ in range(min(n, len(queue))):
                            queue.pop(0)()

                    attn_pair(p, half=0, spread_hook=hook)
                    for u in queue:
                        u()

            # ------- Phases B+C: half-1 attention overlapped with half-0
            # O-projection + FFN, then half-1 O + FFN ---------------------
            with (
                tc.tile_pool(name="accp", bufs=1) as accp,
                tc.tile_pool(name="htp", bufs=1) as htp,
                tc.tile_pool(name="zp", bufs=2) as zp,
                tc.tile_pool(name="w2s", bufs=2) as w2s,
                tc.tile_pool(name="ms", bufs=2) as ms,
                tc.tile_pool(name="xrp", bufs=3) as xrp,
                tc.tile_pool(name="outp", bufs=4) as outp,
            ):
                # y (FFN input / residual): fp8 copy feeds the FFN1 DoubleRow
                # matmuls; bf16 copy feeds the M-path and the final residual
                # add. Per-half tiles, reused across halves (WAR-ordered).
                acc8_sb = accp.tile([P, DC, HQ], fp8)
                acc16_sb = accp.tile([P, DC, HQ], bf16)
                hp8 = htp.tile([P, FC, HQ], fp8)
                htmp = htp.tile([P, GB, HQ], bf16)

                def ffn_units(half):
                    """Yield per-unit closures of O-proj + FFN PE work for one
                    query half. Each unit is ~1-2us of PE work."""
                    q0 = half * HQ
                    # O projection + residual: one unit per feature chunk mo
                    for mo in range(DC):
                        def o_unit(m=mo):
                            xr = xrp.tile([P, HQ], f32, tag="xr")
                            nc.sync.dma_start(xr[:], xres_d[:, m, q0 : q0 + HQ])
                            ps = psA.tile([P, HQ], f32, tag="ps")
                            for c in range(DC2):
                                nc.tensor.matmul(
                                    ps,
                                    wo_sb[:, 2 * c : 2 * c + 2, m * P : (m + 1) * P],
                                    ctxT_sb[:, 2 * c : 2 * c + 2, q0 : q0 + HQ],
                                    start=(c == 0),
                                    stop=(c == DC2 - 1),
                                    perf_mode=DR,
                                )
                            nc.vector.scalar_tensor_tensor(
                                acc16_sb[:, m, :], ps, RWS, xr, MUL, ADD
                            )
                            nc.vector.tensor_scalar_mul(
                                acc8_sb[:, m, :], acc16_sb[:, m, :], 1.0
                            )
                        yield o_unit
                    # FFN layer 1 (full fp8 DR): one unit per ffn chunk m.
                    # z staged bf16 with b1 folded in; gelu runs batched over
                    # GB chunks in ONE instruction — avoiding the 1.3us ACT
                    # table reload that every EXP<->GELU switch costs. The
                    # batch-closing unit emits gelu + the DVE h' = h - ALPHA*z.
                    zbox = []
                    for mf in range(FC):
                        def f1_unit(m=mf, zbox=zbox):
                            if m % GB == 0:
                                zbox[:] = [
                                    zp.tile([P, GB, HQ], bf16, tag="z8", name="z8")
                                ]
                            z8 = zbox[0]
                            ps = psA.tile([P, HQ], f32, tag="ps")
                            for c in range(DC2):
                                nc.tensor.matmul(
                                    ps,
                                    w1_sb[:, m, 2 * c : 2 * c + 2, :],
                                    acc8_sb[:, 2 * c : 2 * c + 2, :],
                                    start=(c == 0),
                                    stop=(c == DC2 - 1),
                                    perf_mode=DR,
                                )
                            nc.vector.tensor_scalar(
                                z8[:, m % GB, :],
                                ps,
                                RWS,
                                b1_sb[:, m : m + 1],
                                MUL,
                                ADD,
                            )
                            if m % GB == GB - 1:
                                m0 = m - GB + 1
                                nc.scalar.activation(htmp[:], z8[:, :, :], AF.Gelu)
                                nc.vector.scalar_tensor_tensor(
                                    hp8[:, m0 : m0 + GB, :],
                                    z8[:, :, :],
                                    -ALPHA,
                                    htmp[:],
                                    MUL,
                                    ADD,
                                )
                        yield f1_unit
                    # FFN layer 2: h'@W2 (fp8 DR) + y@M (bf16) + residual,
                    # three units per output chunk mo sharing one PSUM bank.
                    for mo in range(DC):
                        box = []
                        def f2a(m=mo, box=box):
                            w2t = w2s.tile([P, FC, P], fp8, tag="w2c")
                            nc.sync.dma_start(w2t[:], w2_d[m])
                            mt = ms.tile([P, DC, P], bf16, tag="mc")
                            nc.sync.dma_start(mt[:], m_d[m])
                            ps = psA.tile([P, HQ], f32, tag="ps")
                            box[:] = [w2t, mt, ps]
                            for c in range(FC2 // 2):
                                nc.tensor.matmul(
                                    ps,
                                    w2t[:, 2 * c : 2 * c + 2, :],
                                    hp8[:, 2 * c : 2 * c + 2, :],
                                    start=(c == 0),
                                    stop=False,
                                    perf_mode=DR,
                                )
                        def f2b(box=box):
                            # M-path in the middle unit: it needs only acc16,
                            # buying the last gelu batch's h' one more unit of
                            # latency before f2c reads it
                            w2t, mt, ps = box
                            for k in range(DC):
                                nc.tensor.matmul(
                                    ps,
                                    mt[:, k, :],
                                    acc16_sb[:, k, :],
                                    start=False,
                                    stop=False,
                                )
                        def f2c(m=mo, box=box):
                            w2t, mt, ps = box
                            for c in range(FC2 // 2, FC2):
                                nc.tensor.matmul(
                                    ps,
                                    w2t[:, 2 * c : 2 * c + 2, :],
                                    hp8[:, 2 * c : 2 * c + 2, :],
                                    start=False,
                                    stop=(c == FC2 - 1),
                                    perf_mode=DR,
                                )
                            ot = outp.tile([P, HQ], f32, tag="ot")
                            import os
                            dbg = os.environ.get("KDBG", "")
                            if dbg == "y":  # dump y (acc16)
                                nc.vector.tensor_scalar_mul(
                                    ot, acc16_sb[:, m, :], 1.0
                                )
                            elif dbg == "z":  # dump z chunk m (of first GB)
                                nc.vector.tensor_scalar_mul(
                                    ot, hp8[:, m, :], 1.0
                                )
                            elif dbg == "z2":  # dump h' chunks 8-15
                                nc.vector.tensor_scalar_mul(
                                    ot, hp8[:, m + 8, :], 1.0
                                )
                            elif dbg == "z3":  # dump h' chunks 24-31
                                nc.vector.tensor_scalar_mul(
                                    ot, hp8[:, m + 24, :], 1.0
                                )
                            elif dbg == "ffn1":  # dump h' path only
                                nc.vector.tensor_scalar(
                                    ot, ps, RWS, b2_sb[:, m : m + 1], MUL, ADD
                                )
                            else:
                                nc.vector.tensor_scalar(
                                    ot, ps, RWS, b2_sb[:, m : m + 1], MUL, ADD
                                )
                                nc.vector.tensor_add(
                                    ot, ot, acc16_sb[:, m, :]
                                )
                            nc.sync.dma_start(
                                outT_d[:, m, q0 : q0 + HQ], ot
                            )
                        yield f2a
                        yield f2b
                        yield f2c

                # Phase B: half-1 attention with half-0 O+FFN interleaved.
                units = list(ffn_units(half=0))
                ui = [0]

                def drain(n):
                    for _ in range(n):
                        if ui[0] < len(units):
                            units[ui[0]]()
                            ui[0] += 1

                for p in range(NPAIR):
                    def hook(skh):
                        drain(1)
                    attn_pair(p, half=1, spread_hook=hook)
                drain(len(units))  # leftovers

                # Phase C: half-1 O + FFN, PE-dense
                for u in ffn_units(half=1):
                    u()

    nc.compile()
    return nc


def _get_program():
    if "nc" not in _CACHE:
        _CACHE["nc"] = _build_program()
    return _CACHE["nc"]


def _wlayout(W):
    # [D_in, D_out] -> [P, D_in//P, D_out]
    return np.ascontiguousarray(
        W.reshape(W.shape[0] // P, P, W.shape[1]).transpose(1, 0, 2)
    )


def _blayout(b):
    # [D] -> [P, D//P]
    return np.ascontiguousarray(b.reshape(b.shape[0] // P, P).T)


def prepare_in_maps(x, Wq, bq, Wk, bk, Wv, bv, Wo, bo, W1, b1, W2, b2):
    x = np.asarray(x, np.float32)
    Wq = np.asarray(Wq, np.float32)
    bq = np.asarray(bq, np.float32)
    Wk = np.asarray(Wk, np.float32)
    bk = np.asarray(bk, np.float32)
    Wv = np.asarray(Wv, np.float32)
    bv = np.asarray(bv, np.float32)
    Wo = np.asarray(Wo, np.float32)
    bo = np.asarray(bo, np.float32)
    W1 = np.asarray(W1, np.float32)
    b1 = np.asarray(b1, np.float32)
    W2 = np.asarray(W2, np.float32)
    b2 = np.asarray(b2, np.float32)

    scale = DH ** -0.5
    M = ALPHA * (W1 @ W2)  # merged linear branch of the gelu split
    b2p = b2 + ALPHA * (b1 @ W2)
    def _pairmajor(w):  # [P, DC, D] -> [P, NPAIR, DC, P]
        return np.ascontiguousarray(
            w.reshape(P, DC, NPAIR, P).transpose(0, 2, 1, 3)
        )

    shared = {
        # x8 under the generic x16: wq also carries the 1/sqrt(DH) q scale
        "wq": _pairmajor(_wlayout(Wq * (scale * WS * 8.0))).astype(FP8),
        "wk": _pairmajor(_wlayout(Wk * WS)).astype(FP8),
        "wv": np.ascontiguousarray(
            _wlayout(Wv * WS).reshape(P, DC, 2, 512).transpose(0, 2, 1, 3)
        ).astype(FP8),
        "wo": _wlayout(Wo * (WS / CTXS)).astype(FP8),
        "w1": np.ascontiguousarray(
            (W1 * WS).reshape(DC, P, FC, P).transpose(1, 2, 0, 3)
        ).astype(FP8),
        "w2": np.ascontiguousarray(
            (W2 * WS).reshape(FC, P, DC, P).transpose(2, 1, 0, 3)
        ).astype(FP8),
        # M shares the FFN2 PSUM accumulator with h'@(W2*WS); the readout's
        # RWS applies to the whole bank, so M carries the same WS pre-scale
        "m": np.ascontiguousarray(
            (M * WS).reshape(DC, P, DC, P).transpose(2, 1, 0, 3)
        ).astype(BF16),
        "bq": _blayout(bq * scale),
        "bk": _blayout(bk),
        "bvb": np.ascontiguousarray(np.broadcast_to(bv, (P, D))).astype(BF16),
        "b1": _blayout(b1),
        "b2": _blayout(b2p),
    }

    in_maps = []
    for c in range(NCORES):
        b_idx, half = divmod(c, 2)
        xb = x[b_idx]  # [S, D]
        # rotate tokens so this core's 1024 query tokens are first: the
        # device program always projects Q from xT chunks 0-1 (keys are
        # permuted identically in kt/v; softmax is key-order invariant)
        xbT = np.roll(xb.T, -half * SQ, axis=1)  # [D, S]
        xT = np.ascontiguousarray(
            xbT.reshape(DC, P, 4, 512).transpose(1, 2, 0, 3)
        ).astype(FP8)
        xres = np.ascontiguousarray(
            (xbT[:, 0:SQ] + bo[:, None]).reshape(DC, P, SQ).transpose(1, 0, 2)
        ).astype(np.float32)
        in_maps.append(dict(shared, xT=xT, xres=xres))
    return in_maps


def assemble_out(results):
    out = np.empty((B, S, D), np.float32)
    for c in range(NCORES):
        b_idx, half = divmod(c, 2)
        outT = results[c]["outT"]  # [P, DC, SQ]
        out[b_idx, half * SQ : (half + 1) * SQ] = (
            outT.transpose(1, 0, 2).reshape(D, SQ).T
        )
    return out


def kernel(**inputs):
    from concourse.bass_utils import run_bass_kernel_spmd

    in_maps = prepare_in_maps(**inputs)
    nc = _get_program()
    res = run_bass_kernel_spmd(nc, in_maps, core_ids=list(range(NCORES)))
    return assemble_out(res.results)


# revision 29
# speedup vs baseline: 1.0003x; 1.0003x over previous
"""Trainium2 Bass kernel for a dense transformer layer (attention + FFN).

Sharding: 8 shards = (batch b, sequence half) pairs. Each core computes the
full K/V projections for its batch (2x redundant) and Q/attention/FFN for its
1024-token query slice. No cross-core communication.

On-device layout is feature-major (transposed): activations live as
[feature, token] so every matmul is lhsT.T @ rhs with natural weight layouts.

Precision: QKV/O projections, the attention ctx matmuls, and BOTH FFN
matmuls run fp8e4 with DoubleRow perf mode (256-row contraction per
instruction, ~1.9x PE speedup). The FFN stays inside the 2e-2 error gate
via a gelu linear-split: gelu(z) = ALPHA*z + h'(z) with h' = gelu - ALPHA*z.
The ALPHA*z branch is exact linear algebra folded into a host-precomputed
merged matrix M = ALPHA*W1@W2 applied to y in bf16 (8 single-row matmuls
per output chunk); only the small-magnitude h' branch runs fp8, cutting the
fp8 FFN2 quantization error ~3x (sim: full-fp8 direct 2.5e-2; split 1.5e-2).
Weights are pre-scaled x16 on the host to sit in fp8e4's normal range; the
1/16 is folded into the PSUM->SBUF readout ops. ctx is scaled x32 via the
softmax reciprocal (compensated in Wo). fp32 PSUM accumulation; residual
stream held bf16 (acc16).

Schedule: the query slice is split in two 512-token halves. Phase A runs
K/Q/V projections and half-0 attention (ACT-bound on exp). Phase B runs
half-1 attention on ACT while the PE stream interleaves half-0's O
projection and FFN between attention matmuls — keeping the PE array dense
(avoids HAM down-throttle) and overlapping the exp floor with FFN compute.
Phase C finishes half-1's O projection and FFN. W1 (4MB fp8) is resident in
SBUF from phase A; W2/M stream in small fp8/bf16 chunks.
"""

import numpy as np
import ml_dtypes

B, S, D = 4, 2048, 1024
H, DH, F = 16, 64, 4096
P = 128
NCORES = 8
SQ = B * S // NCORES  # 1024 query tokens per core
HQ = SQ // 2  # 512-token query half
DC = D // P  # 8 feature chunks
DC2 = DC // 2  # 4 double chunks (DoubleRow)
FC = F // P  # 32 ffn chunks
FC2 = FC // 2
SKC = S // P  # 16 key chunks
SKC2 = SKC // 2
NPAIR = H // 2  # 8 head pairs (2 heads per 128-feature chunk)
GB = 4  # gelu batch (chunks per ACT tanh instruction)

WS = 16.0  # host-side fp8 weight scale
RWS = 1.0 / WS
CTXS = 32.0  # ctx fp8 scale (folded into softmax recip; compensated in Wo)
ALPHA = 0.6  # gelu linear-split coefficient (min error on this data)

BF16 = ml_dtypes.bfloat16
FP8 = ml_dtypes.float8_e4m3

_CACHE = {}


def _build_program():
    import concourse.mybir as mybir
    import concourse.tile as tile
    from concourse import bacc

    f32 = mybir.dt.float32
    bf16 = mybir.dt.bfloat16
    fp8 = mybir.dt.float8e4
    AF = mybir.ActivationFunctionType
    DR = mybir.MatmulPerfMode.DoubleRow
    MUL = mybir.AluOpType.mult
    ADD = mybir.AluOpType.add

    nc = bacc.Bacc("TRN2", target_bir_lowering=False, debug=False, num_devices=NCORES)

    # xT/wv/wk/wq are chunk-major so every startup DMA moves contiguous
    # 1-4KB per-partition lines (small strided lines gut DMA throughput)
    xT_d = nc.dram_tensor("xT", [P, 4, DC, 512], fp8, kind="ExternalInput")
    xres_d = nc.dram_tensor("xres", [P, DC, SQ], f32, kind="ExternalInput")
    wq_d = nc.dram_tensor("wq", [P, NPAIR, DC, P], fp8, kind="ExternalInput")
    wk_d = nc.dram_tensor("wk", [P, NPAIR, DC, P], fp8, kind="ExternalInput")
    wv_d = nc.dram_tensor("wv", [P, 2, DC, 512], fp8, kind="ExternalInput")
    wo_d = nc.dram_tensor("wo", [P, DC, D], fp8, kind="ExternalInput")
    w1_d = nc.dram_tensor("w1", [P, FC, DC, P], fp8, kind="ExternalInput")
    w2_d = nc.dram_tensor("w2", [P, DC, FC, P], fp8, kind="ExternalInput")
    m_d = nc.dram_tensor("m", [DC, P, DC, P], bf16, kind="ExternalInput")
    bq_d = nc.dram_tensor("bq", [P, DC], f32, kind="ExternalInput")
    bk_d = nc.dram_tensor("bk", [P, DC], f32, kind="ExternalInput")
    bvb_d = nc.dram_tensor("bvb", [P, D], bf16, kind="ExternalInput")
    b1_d = nc.dram_tensor("b1", [P, FC], f32, kind="ExternalInput")
    b2_d = nc.dram_tensor("b2", [P, DC], f32, kind="ExternalInput")
    outT_d = nc.dram_tensor("outT", [P, DC, SQ], f32, kind="ExternalOutput")

    with tile.TileContext(nc) as tc:
        with (
            tc.tile_pool(name="psA", bufs=2, space="PSUM") as psA,
            tc.tile_pool(name="psS", bufs=2, space="PSUM") as psS,
            tc.tile_pool(name="psC", bufs=2, space="PSUM") as psC,
            tc.tile_pool(name="biasp", bufs=1) as biasp,
            tc.tile_pool(name="ctxp", bufs=1) as ctxp,
            tc.tile_pool(name="ep", bufs=6) as ep,
            tc.tile_pool(name="rp", bufs=2) as rp,
            tc.tile_pool(name="rbp", bufs=2) as rbp,
        ):
            bq_sb = biasp.tile([P, DC], f32)
            bk_sb = biasp.tile([P, DC], f32)
            b1_sb = biasp.tile([P, FC], f32)
            b2_sb = biasp.tile([P, DC], f32)
            nc.scalar.dma_start(bq_sb[:], bq_d[:])
            nc.scalar.dma_start(bk_sb[:], bk_d[:])
            nc.scalar.dma_start(b1_sb[:], b1_d[:])
            nc.scalar.dma_start(b2_sb[:], b2_d[:])

            ctxT_sb = ctxp.tile([P, DC, SQ], fp8)
            wo_sb = ctxp.tile([P, DC, D], fp8)
            w1_sb = ctxp.tile([P, FC, DC, P], fp8)
            w2_sb = ctxp.tile([P, DC, FC, P], fp8)
            v_sb = ctxp.tile([P, SKC, H, DH + 1], fp8)
            kt_all = ctxp.tile([P, NPAIR, S], fp8)
            qt_all = ctxp.tile([P, NPAIR, SQ], fp8)

            def attn_pair(p, half, spread_hook=None):
                """Attention for head pair (2p, 2p+1), queries
                [half*HQ, half*HQ+HQ). spread_hook(skh) emits filler PE work.

                Software-pipelined: scores run one sk-chunk-pair ahead of ctx
                so the in-order PE stream never serializes the next scores
                behind exp — ACT stays continuously busy on exp."""
                q0 = half * HQ
                pc0 = psC.tile([P, HQ], f32, tag="pc")
                pc1 = psC.tile([P, HQ], f32, tag="pc")
                E2s = [None] * SKC2

                def emit_scores(skh):
                    # E2: exp(scores), laid [key, chunk-parity, headA|headB]
                    # = the ctx DoubleRow moving operand.
                    E2 = ep.tile([P, 2, 2 * HQ], fp8)
                    E2s[skh] = E2
                    for hs in range(2):
                        sk = 2 * skh + hs
                        ss = psS.tile([P, 2 * HQ], f32)
                        nc.tensor.matmul(
                            ss[:, 0:HQ],
                            kt_all[0:64, p, sk * P : (sk + 1) * P],
                            qt_all[0:64, p, q0 : q0 + HQ],
                            start=True,
                            stop=True,
                        )
                        nc.tensor.matmul(
                            ss[:, HQ : 2 * HQ],
                            kt_all[64:128, p, sk * P : (sk + 1) * P],
                            qt_all[64:128, p, q0 : q0 + HQ],
                            start=True,
                            stop=True,
                        )
                        nc.scalar.activation(E2[:, hs, :], ss, AF.Exp)

                emit_scores(0)
                for skh in range(SKC2):
                    if skh + 1 < SKC2:
                        emit_scores(skh + 1)
                    # filler PE work lands between next-scores and this ctx so
                    # the PE covers the exp latency instead of stalling on E2
                    if spread_hook is not None:
                        spread_hook(skh)
                    E2 = E2s[skh]
                    nc.tensor.matmul(
                        pc0[:65],
                        v_sb[:, 2 * skh : 2 * skh + 2, 2 * p, :],
                        E2[:, :, 0:HQ],
                        start=(skh == 0),
                        stop=(skh == SKC2 - 1),
                        perf_mode=DR,
                    )
                    nc.tensor.matmul(
                        pc1[:65],
                        v_sb[:, 2 * skh : 2 * skh + 2, 2 * p + 1, :],
                        E2[:, :, HQ : 2 * HQ],
                        start=(skh == 0),
                        stop=(skh == SKC2 - 1),
                        perf_mode=DR,
                    )
                # softmax normalization: ctx * (CTXS / rowsum); the CTXS fp8
                # range scale is divided back out in Wo. (approx recip is ~18
                # correct bits, plenty for a softmax denom)
                for hh, pc in ((0, pc0), (1, pc1)):
                    s0 = rp.tile([1, HQ], f32, tag="s")
                    nc.vector.tensor_scalar_mul(s0, pc[64:65, :], 1.0 / CTXS)
                    r0 = rp.tile([1, HQ], f32, tag="r")
                    nc.vector.reciprocal_approx_fast(r0, s0)
                    rb0 = rbp.tile([64, HQ], f32, tag="rb")
                    nc.gpsimd.partition_broadcast(rb0, r0)
                    nc.vector.tensor_mul(
                        ctxT_sb[64 * hh : 64 * hh + 64, p, q0 : q0 + HQ],
                        pc[0:64, :],
                        rb0,
                    )

            # ---------------- Phase A: projections + half-0 attention -------
            with (
                tc.tile_pool(name="abp", bufs=1) as abp,
                tc.tile_pool(name="wvp", bufs=1) as wvp,
                tc.tile_pool(name="ws", bufs=3) as ws,
            ):
                # x^T in 4 column-chunk tiles so V/K matmuls start after the
                # first chunk lands rather than after the full DMA.
                xTs = [
                    abp.tile([P, DC, 512], fp8, tag=f"xT{c}", name=f"xT{c}")
                    for c in range(4)
                ]
                wvs = [
                    wvp.tile([P, DC, 512], fp8, tag=f"wv{c}", name=f"wv{c}")
                    for c in range(2)
                ]
                bvb_sb = abp.tile([P, D], bf16)
                # startup DMA priority comes from in-queue FIFO order: the
                # first V matmul's inputs (x and wv chunks 0-1) lead their
                # queues; bulk transfers follow behind them, spread over the
                # sync/gpsimd/scalar queues.
                nc.sync.dma_start(xTs[0][:, 0:2, :], xT_d[:, 0, 0:2])
                nc.gpsimd.dma_start(wvs[0][:, 0:2, :], wv_d[:, 0, 0:2])
                nc.sync.dma_start(xTs[0][:, 2:DC, :], xT_d[:, 0, 2:DC])
                nc.gpsimd.dma_start(wvs[0][:, 2:DC, :], wv_d[:, 0, 2:DC])
                nc.scalar.dma_start(bvb_sb[:], bvb_d[:])
                nc.sync.dma_start(xTs[1][:], xT_d[:, 1])
                nc.scalar.dma_start(xTs[2][:], xT_d[:, 2])
                nc.sync.dma_start(xTs[3][:], xT_d[:, 3])
                nc.gpsimd.dma_start(wvs[1][:], wv_d[:, 1])
                nc.scalar.dma_start(wo_sb[:], wo_d[:])
                # resident full-fp8 W1+W2 (8MB) in big contiguous DMAs on
                # the gpsimd queue (NOT scalar: scalar-queue DMA triggers
                # would steal ACT cycles from the exp stream)
                nc.gpsimd.dma_start(w1_sb[:, 0 : FC // 2], w1_d[:, 0 : FC // 2])
                nc.gpsimd.dma_start(w1_sb[:, FC // 2 :], w1_d[:, FC // 2 :])
                nc.gpsimd.dma_start(w2_sb[:, 0 : DC // 2], w2_d[:, 0 : DC // 2])
                nc.gpsimd.dma_start(w2_sb[:, DC // 2 :], w2_d[:, DC // 2 :])

                # V projection, token-major: v[sk, dv] (+ ones column per
                # head). fp8: it is the ctx DoubleRow stationary operand.
                nc.vector.memset(v_sb[:, :, :, DH : DH + 1], 1.0)

                def emit_v(nv, sks, h0=0, h1=8):
                    nh = h1 - h0
                    for sk in sks:
                        xt = xTs[sk // 4]
                        co = (sk % 4) * P
                        ps = psA.tile([P, 512], f32, tag="ps")
                        for c in range(DC2):
                            nc.tensor.matmul(
                                ps[:, : nh * DH],
                                xt[:, 2 * c : 2 * c + 2, co : co + P],
                                wvs[nv][:, 2 * c : 2 * c + 2, h0 * DH : h1 * DH],
                                start=(c == 0),
                                stop=(c == DC2 - 1),
                                perf_mode=DR,
                            )
                        nc.vector.scalar_tensor_tensor(
                            v_sb[:, sk, nv * 8 + h0 : nv * 8 + h1, 0:DH],
                            ps[:, : nh * DH].rearrange("p (h d) -> p h d", h=nh),
                            RWS,
                            bvb_sb[
                                :, nv * 512 + h0 * DH : nv * 512 + h1 * DH
                            ].rearrange("p (h d) -> p h d", h=nh),
                            MUL,
                            ADD,
                        )

                def kq_units(p):
                    """K/Q projection PE work for pair p as 6 ~1-1.5us units.
                    Weight DMAs are issued at queue-build time (prefetch)."""
                    wkt = ws.tile([P, DC, P], fp8, tag="wchunk")
                    nc.sync.dma_start(wkt[:], wk_d[:, p])
                    wqt = ws.tile([P, DC, P], fp8, tag="wchunk")
                    nc.sync.dma_start(wqt[:], wq_d[:, p])
                    units = []
                    for n in range(S // 512):
                        def ku(n=n):
                            ps = psA.tile([P, 512], f32, tag="ps")
                            for c in range(DC2):
                                nc.tensor.matmul(
                                    ps,
                                    wkt[:, 2 * c : 2 * c + 2, :],
                                    xTs[n][:, 2 * c : 2 * c + 2, :],
                                    start=(c == 0),
                                    stop=(c == DC2 - 1),
                                    perf_mode=DR,
                                )
                            nc.vector.tensor_scalar(
                                kt_all[:, p, n * 512 : (n + 1) * 512],
                                ps,
                                RWS,
                                bk_sb[:, p : p + 1],
                                MUL,
                                ADD,
                            )
                        units.append(ku)
                    # wq is x16 overall on the host (x128 on Wq*scale for fp8
                    # range); divide the full 128 back out in the readout.
                    # xT is host-rotated per core so its own 1024 query
                    # tokens are chunks 0-1 (keys permuted; softmax invariant)
                    for n in range(SQ // 512):
                        def qu(n=n):
                            ps = psA.tile([P, 512], f32, tag="ps")
                            for c in range(DC2):
                                nc.tensor.matmul(
                                    ps,
                                    wqt[:, 2 * c : 2 * c + 2, :],
                                    xTs[n][:, 2 * c : 2 * c + 2, :],
                                    start=(c == 0),
                                    stop=(c == DC2 - 1),
                                    perf_mode=DR,
                                )
                            nc.vector.tensor_scalar(
                                qt_all[:, p, n * 512 : (n + 1) * 512],
                                ps,
                                RWS / 8.0,
                                bq_sb[:, p : p + 1],
                                MUL,
                                ADD,
                            )
                        units.append(qu)
                    return units

                # V(nv=1) chunk counts per pair (heads 8-15, needed from
                # pair 4 on — must complete by end of pair 3)
                V1_PLAN = {0: 2, 1: 5, 2: 5, 3: 4}

                emit_v(0, range(SKC))
                for u in kq_units(0):
                    u()
                v1_next = 0
                for p in range(NPAIR):
                    queue = []
                    if p + 1 < NPAIR:
                        queue.extend(kq_units(p + 1))
                    for _ in range(V1_PLAN.get(p, 0)):
                        queue.append(lambda sk=v1_next: emit_v(1, [sk]))
                        v1_next += 1

                    def hook(skh, queue=queue):
                        # drain >=1 unit/slot, catching up so the queue
                        # empties by the last slot of the pair
                        rem_slots = SKC2 - skh
                        n = max(1, -(-len(queue) // rem_slots))
                        for _ in range(min(n, len(queue))):
                            queue.pop(0)()

                    attn_pair(p, half=0, spread_hook=hook)
                    for u in queue:
                        u()

            # ------- Phases B+C: half-1 attention overlapped with half-0
            # O-projection + FFN, then half-1 O + FFN ---------------------
            with (
                tc.tile_pool(name="accp", bufs=1) as accp,
                tc.tile_pool(name="htp", bufs=1) as htp,
                tc.tile_pool(name="zp", bufs=2) as zp,
                tc.tile_pool(name="ms", bufs=2) as ms,
                tc.tile_pool(name="xrp", bufs=3) as xrp,
                tc.tile_pool(name="outp", bufs=4) as outp,
            ):
                # y (FFN input / residual): fp8 copy feeds the FFN1 DoubleRow
                # matmuls; bf16 copy feeds the M-path and the final residual
                # add. Per-half tiles, reused across halves (WAR-ordered).
                acc8_sb = accp.tile([P, DC, HQ], fp8)
                acc16_sb = accp.tile([P, DC, HQ], bf16)
                hp8 = htp.tile([P, FC, HQ], fp8)
                ttile = htp.tile([P, GB, HQ], bf16)
                utile = htp.tile([P, GB, HQ], bf16)

                def ffn_units(half):
                    """Yield per-unit closures of O-proj + FFN PE work for one
                    query half. Each unit is ~1-2us of PE work."""
                    q0 = half * HQ
                    # O projection + residual: one unit per feature chunk mo
                    for mo in range(DC):
                        def o_unit(m=mo):
                            xr = xrp.tile([P, HQ], f32, tag="xr")
                            nc.sync.dma_start(xr[:], xres_d[:, m, q0 : q0 + HQ])
                            ps = psA.tile([P, HQ], f32, tag="ps")
                            for c in range(DC2):
                                nc.tensor.matmul(
                                    ps,
                                    wo_sb[:, 2 * c : 2 * c + 2, m * P : (m + 1) * P],
                                    ctxT_sb[:, 2 * c : 2 * c + 2, q0 : q0 + HQ],
                                    start=(c == 0),
                                    stop=(c == DC2 - 1),
                                    perf_mode=DR,
                                )
                            nc.vector.scalar_tensor_tensor(
                                acc16_sb[:, m, :], ps, RWS, xr, MUL, ADD
                            )
                            nc.vector.tensor_scalar_mul(
                                acc8_sb[:, m, :], acc16_sb[:, m, :], 1.0
                            )
                        yield o_unit
                    # FFN layer 1 (full fp8 DR): one unit per ffn chunk m.
                    # z staged bf16 with b1 folded in. The batch-closing unit
                    # emits the sigmoid-gelu: h = z*sigmoid(1.702z) =
                    # z*(0.5*tanh(0.851z)+0.5), so h' = h - ALPHA*z =
                    # z*(0.5*T + 0.5 - ALPHA). Tanh lives in the SAME ACT
                    # table as Exp — zero table reloads, and small GB keeps
                    # ACT bursts short so the exp stream never backs up the
                    # scores pipeline (PSUM ss WAR).
                    zbox = []
                    for mf in range(FC):
                        def f1_unit(m=mf, zbox=zbox):
                            if m % GB == 0:
                                zbox[:] = [
                                    zp.tile([P, GB, HQ], bf16, tag="z8", name="z8")
                                ]
                            z8 = zbox[0]
                            ps = psA.tile([P, HQ], f32, tag="ps")
                            for c in range(DC2):
                                nc.tensor.matmul(
                                    ps,
                                    w1_sb[:, m, 2 * c : 2 * c + 2, :],
                                    acc8_sb[:, 2 * c : 2 * c + 2, :],
                                    start=(c == 0),
                                    stop=(c == DC2 - 1),
                                    perf_mode=DR,
                                )
                            nc.vector.tensor_scalar(
                                z8[:, m % GB, :],
                                ps,
                                RWS,
                                b1_sb[:, m : m + 1],
                                MUL,
                                ADD,
                            )
                            if m % GB == GB - 1:
                                m0 = m - GB + 1
                                nc.scalar.activation(
                                    ttile[:], z8[:, :, :], AF.Tanh, scale=0.851
                                )
                                nc.vector.tensor_scalar(
                                    utile[:], ttile[:], 0.5, 0.5 - ALPHA, MUL, ADD
                                )
                                nc.vector.tensor_mul(
                                    hp8[:, m0 : m0 + GB, :], utile[:], z8[:, :, :]
                                )
                        yield f1_unit
                    # FFN layer 2: h'@W2 (fp8 DR) + y@M (bf16) + residual,
                    # three units per output chunk mo sharing one PSUM bank.
                    for mo in range(DC):
                        box = []
                        def f2a(m=mo, box=box):
                            mt = ms.tile([P, DC, P], bf16, tag="mc")
                            nc.sync.dma_start(mt[:], m_d[m])
                            ps = psA.tile([P, HQ], f32, tag="ps")
                            box[:] = [mt, ps]
                            for c in range(FC2 // 2):
                                nc.tensor.matmul(
                                    ps,
                                    w2_sb[:, m, 2 * c : 2 * c + 2, :],
                                    hp8[:, 2 * c : 2 * c + 2, :],
                                    start=(c == 0),
                                    stop=False,
                                    perf_mode=DR,
                                )
                        def f2b(box=box):
                            # M-path in the middle unit: it needs only acc16,
                            # buying the last gelu batch's h' one more unit of
                            # latency before f2c reads it
                            mt, ps = box
                            for k in range(DC):
                                nc.tensor.matmul(
                                    ps,
                                    mt[:, k, :],
                                    acc16_sb[:, k, :],
                                    start=False,
                                    stop=False,
                                )
                        def f2c(m=mo, box=box):
                            mt, ps = box
                            for c in range(FC2 // 2, FC2):
                                nc.tensor.matmul(
                                    ps,
                                    w2_sb[:, m, 2 * c : 2 * c + 2, :],
                                    hp8[:, 2 * c : 2 * c + 2, :],
                                    start=False,
                                    stop=(c == FC2 - 1),
                                    perf_mode=DR,
                                )
                            ot = outp.tile([P, HQ], f32, tag="ot")
                            nc.vector.tensor_scalar(
                                ot, ps, RWS, b2_sb[:, m : m + 1], MUL, ADD
                            )
                            nc.vector.tensor_add(
                                ot, ot, acc16_sb[:, m, :]
                            )
                            nc.sync.dma_start(
                                outT_d[:, m, q0 : q0 + HQ], ot
                            )
                        yield f2a
                        yield f2b
                        yield f2c

                # Phase B: half-1 attention with half-0 O+FFN interleaved.
                units = list(ffn_units(half=0))
                ui = [0]

                def drain(n):
                    for _ in range(n):
                        if ui[0] < len(units):
                            units[ui[0]]()
                            ui[0] += 1

                for p in range(NPAIR):
                    def hook(skh):
                        drain(1)
                    attn_pair(p, half=1, spread_hook=hook)
                drain(len(units))  # leftovers

                # Phase C: half-1 O + FFN, PE-dense
                for u in ffn_units(half=1):
                    u()

    nc.compile()
    return nc


def _get_program():
    if "nc" not in _CACHE:
        _CACHE["nc"] = _build_program()
    return _CACHE["nc"]


def _wlayout(W):
    # [D_in, D_out] -> [P, D_in//P, D_out]
    return np.ascontiguousarray(
        W.reshape(W.shape[0] // P, P, W.shape[1]).transpose(1, 0, 2)
    )


def _blayout(b):
    # [D] -> [P, D//P]
    return np.ascontiguousarray(b.reshape(b.shape[0] // P, P).T)


def prepare_in_maps(x, Wq, bq, Wk, bk, Wv, bv, Wo, bo, W1, b1, W2, b2):
    x = np.asarray(x, np.float32)
    Wq = np.asarray(Wq, np.float32)
    bq = np.asarray(bq, np.float32)
    Wk = np.asarray(Wk, np.float32)
    bk = np.asarray(bk, np.float32)
    Wv = np.asarray(Wv, np.float32)
    bv = np.asarray(bv, np.float32)
    Wo = np.asarray(Wo, np.float32)
    bo = np.asarray(bo, np.float32)
    W1 = np.asarray(W1, np.float32)
    b1 = np.asarray(b1, np.float32)
    W2 = np.asarray(W2, np.float32)
    b2 = np.asarray(b2, np.float32)

    scale = DH ** -0.5
    M = ALPHA * (W1 @ W2)  # merged linear branch of the gelu split
    b2p = b2 + ALPHA * (b1 @ W2)
    def _pairmajor(w):  # [P, DC, D] -> [P, NPAIR, DC, P]
        return np.ascontiguousarray(
            w.reshape(P, DC, NPAIR, P).transpose(0, 2, 1, 3)
        )

    shared = {
        # x8 under the generic x16: wq also carries the 1/sqrt(DH) q scale
        "wq": _pairmajor(_wlayout(Wq * (scale * WS * 8.0))).astype(FP8),
        "wk": _pairmajor(_wlayout(Wk * WS)).astype(FP8),
        "wv": np.ascontiguousarray(
            _wlayout(Wv * WS).reshape(P, DC, 2, 512).transpose(0, 2, 1, 3)
        ).astype(FP8),
        "wo": _wlayout(Wo * (WS / CTXS)).astype(FP8),
        "w1": np.ascontiguousarray(
            (W1 * WS).reshape(DC, P, FC, P).transpose(1, 2, 0, 3)
        ).astype(FP8),
        "w2": np.ascontiguousarray(
            (W2 * WS).reshape(FC, P, DC, P).transpose(1, 2, 0, 3)
        ).astype(FP8),
        # M shares the FFN2 PSUM accumulator with h'@(W2*WS); the readout's
        # RWS applies to the whole bank, so M carries the same WS pre-scale
        "m": np.ascontiguousarray(
            (M * WS).reshape(DC, P, DC, P).transpose(2, 1, 0, 3)
        ).astype(BF16),
        "bq": _blayout(bq * scale),
        "bk": _blayout(bk),
        "bvb": np.ascontiguousarray(np.broadcast_to(bv, (P, D))).astype(BF16),
        "b1": _blayout(b1),
        "b2": _blayout(b2p),
    }

    in_maps = []
    for c in range(NCORES):
        b_idx, half = divmod(c, 2)
        xb = x[b_idx]  # [S, D]
        # rotate tokens so this core's 1024 query tokens are first: the
        # device program always projects Q from xT chunks 0-1 (keys are
        # permuted identically in kt/v; softmax is key-order invariant)
        xbT = np.roll(xb.T, -half * SQ, axis=1)  # [D, S]
        xT = np.ascontiguousarray(
            xbT.reshape(DC, P, 4, 512).transpose(1, 2, 0, 3)
        ).astype(FP8)
        xres = np.ascontiguousarray(
            (xbT[:, 0:SQ] + bo[:, None]).reshape(DC, P, SQ).transpose(1, 0, 2)
        ).astype(np.float32)
        in_maps.append(dict(shared, xT=xT, xres=xres))
    return in_maps


def assemble_out(results):
    out = np.empty((B, S, D), np.float32)
    for c in range(NCORES):
        b_idx, half = divmod(c, 2)
        outT = results[c]["outT"]  # [P, DC, SQ]
        out[b_idx, half * SQ : (half + 1) * SQ] = (
            outT.transpose(1, 0, 2).reshape(D, SQ).T
        )
    return out


def kernel(**inputs):
    from concourse.bass_utils import run_bass_kernel_spmd

    in_maps = prepare_in_maps(**inputs)
    nc = _get_program()
    res = run_bass_kernel_spmd(nc, in_maps, core_ids=list(range(NCORES)))
    return assemble_out(res.results)
